# revision 31
# baseline (speedup 1.0000x reference)
"""Chamfer-loss Trainium2 kernel.

kernel(coarse, fine, gt, alpha) -> (loss, loss_coarse, loss_fine)

Strategy: data-parallel over batch (B=8) across the 8 NeuronCores; each core
computes, for its batch element, the two directed chamfer sums for
fine<->gt and coarse<->gt by brute-force pairwise squared distances:

  -d(x,y) = 2*x.y - |x|^2 - |y|^2  is computed directly by the PE array as a
  K=5 matmul:  lhsT = [x0;x1;x2;-|x|^2;-1] (stationary, [5,128] per x-tile),
  rhs = [2*y0;2*y1;2*y2;1;|y|^2] ([5,512] slices).  PSUM then holds -d, so
  "min distance" everywhere becomes "max of -d".

  Row direction (per-x min over y): fused DVE tensor_tensor_reduce per PSUM
  group: accum_out = running max over the free axis (chained via the scalar
  operand); its elementwise output writes an fp16 copy of -d to SBUF.
  Column direction (per-y min over x): fp16 tensor_tensor max accumulation
  (2x DVE mode) into a [128, Ng] accumulator, partition-reduced at the end
  via PE transposes + free-axis reduce.

Host side only shards inputs, averages the per-core sums and applies alpha.
"""

import os
import sys
import numpy as np

sys.path.insert(0, "/opt/trn_rl_repo")

from contextlib import ExitStack

import concourse.bass as bass
import concourse.tile as tile
from concourse import mybir

FP32 = mybir.dt.float32
FP16 = mybir.dt.float16
AX = mybir.AxisListType
OP = mybir.AluOpType

NEG = -1.0e30
NEGF16 = -60000.0

# full-problem shapes (hardcoded; kernel.py must be self-contained)
B, NC, NF, NG = 8, 1024, 8192, 8192

LAST_EXEC_NS = None  # stashed HW exec time from the most recent traced run


def cap_sync_waits(nc):
    """This walrus build accepts only ONE sync wait per instruction.

    Compute-engine instructions: move overflow waits onto injected
    same-engine NoOps (sequencer FIFO preserves ordering).
    DMA instructions (separate queue processors -- a sequencer NoOp does
    NOT gate them): move ALL waits onto a Pool-engine NoOp chain whose
    last link increments a fresh auxiliary semaphore; the DMA then waits
    only on that semaphore.
    """
    used = set()
    for bb in nc.main_func.blocks:
        for ins in bb.instructions:
            si = ins.sync_info
            if si is not None:
                for w in si.on_wait or []:
                    used.add(w.id)
                for u in si.on_update or []:
                    used.add(u.id)
    aux = None
    for i in range(64):
        h = nc.alloc_semaphore(f"capw_aux{i}")
        if h.num not in used:
            aux = h
            break
    assert aux is not None, "no free semaphore for cap_sync_waits"
    aux_count = 0
    n_new = 0
    nid = [0]

    def mknop(engine, wait, update=None):
        nid[0] += 1
        nop = mybir.InstNoOp(name=f"capw-{nid[0]}", ins=[], outs=[])
        nop.engine = engine
        nop.sync_info = mybir.SyncInfo(
            on_wait=[wait] if wait is not None else [],
            on_update=[update] if update is not None else [],
        )
        nc.register_instruction(nop, overwrite=True)
        return nop

    for bb in nc.main_func.blocks:
        out = []
        changed = False
        for ins in bb.instructions:
            si = ins.sync_info
            waits = list(si.on_wait) if (si is not None and si.on_wait) else []
            if len(waits) > 1:
                changed = True
                is_dma = getattr(ins, "queue", None) is not None
                if is_dma:
                    aux_count += 1
                    for i, w in enumerate(waits):
                        upd = (
                            mybir.SyncUpdate(
                                sync_type="semaphore",
                                id=aux.num,
                                ant_name="capw_aux",
                                update_mode="sem-inc",
                                update_value=1,
                                update_reg=None,
                            )
                            if i == len(waits) - 1
                            else None
                        )
                        out.append(mknop(mybir.EngineType.Pool, w, upd))
                        n_new += 1
                    si.on_wait = [
                        mybir.SyncWait(
                            sync_type="semaphore",
                            id=aux.num,
                            ant_name="capw_aux",
                            wait_mode="sem-ge-imm",
                            wait_value=aux_count,
                            wait_reg=None,
                        )
                    ]
                else:
                    for w in waits[:-1]:
                        out.append(mknop(ins.engine, w))
                        n_new += 1
                    si.on_wait = waits[-1:]
            out.append(ins)
        if changed:
            bb.instructions = out
    return n_new


def emit_chamfer(nc, Nf, Ng, Ncs, group=2048):
    """Emit the full per-core program. Dims must divide (128, group)."""
    assert Ng % group == 0 and Nf % 128 == 0 and Ncs % 128 == 0
    n_groups = Ng // group
    mm_n = 512
    assert group % mm_n == 0

    fine_d = nc.dram_tensor("fineT", [3, Nf], FP32, kind="ExternalInput")
    gt_d = nc.dram_tensor("gt", [3, Ng], FP32, kind="ExternalInput")
    coarse_d = nc.dram_tensor("coarseT", [3, Ncs], FP32, kind="ExternalInput")
    ident_d = nc.dram_tensor("ident", [128, 128], FP16, kind="ExternalInput")
    # consts[0] = +1.0 row, consts[1] = -1.0 row (engine ops can't write
    # partitions 3/4 directly: partition base must be 0/32/64/96)
    consts_d = nc.dram_tensor("consts", [2, max(Nf, Ng)], FP32, kind="ExternalInput")
    out_d = nc.dram_tensor("out", [1, 4], FP32, kind="ExternalOutput")

    with ExitStack() as ctx:
        tc = ctx.enter_context(tile.TileContext(nc))
        const = ctx.enter_context(tc.tile_pool(name="const", bufs=1))
        lr = ctx.enter_context(tc.tile_pool(name="lr", bufs=1))
        accp = ctx.enter_context(tc.tile_pool(name="accp", bufs=1))
        tmpp = ctx.enter_context(tc.tile_pool(name="tmpp", bufs=3))
        scrp = ctx.enter_context(tc.tile_pool(name="scrp", bufs=3))
        smallp = ctx.enter_context(tc.tile_pool(name="smallp", bufs=4))

        ident = const.tile([128, 128], FP16)
        nc.sync.dma_start(ident[:], ident_d[:, :])
        out_sb = const.tile([1, 4], FP32)

        # ---- build R = [y0;y1;y2; 1; |y|^2]  [5, Ng] ----
        # |y|^2 computed in a [128, Ng/128] block layout ([p,c] = y_{c*128+p}),
        # then DMA'd into partition 4 (engine ops can't start at partition 4).
        # R/L are allocated [128, N]: the K=5 operand rows are replicated at
        # base partitions 0/32/64/96 so four row-group-packed matmuls can run
        # concurrently in the PE array (K=5 uses only 5 of 128 rows).
        R = lr.tile([128, Ng], FP32)
        nc.sync.dma_start(R[0:3, :], gt_d[:, :])
        nc.scalar.mul(R[0:3, :], R[0:3, :], 2.0)
        nc.sync.dma_start(R[3:4, :], consts_d[0:1, 0:Ng])
        Cg = Ng // 128
        yb = tmpp.tile([128, 3 * Cg], FP32, tag="yb")
        # [p, k, c] = y-coord k of point p*Cg+c  (p-major point blocks)
        nc.sync.dma_start(yb[:], gt_d.rearrange("k (p c) -> p k c", p=128))
        nc.scalar.activation(yb[:], yb[:], mybir.ActivationFunctionType.Square)
        ysum = tmpp.tile([128, Cg], FP32, tag="ysum")
        ybv = yb.rearrange("p (k c) -> p k c", k=3)
        nc.vector.tensor_tensor(ysum[:], ybv[:, 0, :], ybv[:, 1, :], op=OP.add)
        nc.vector.tensor_tensor(ysum[:], ysum[:], ybv[:, 2, :], op=OP.add)
        nc.sync.dma_start(R[4:5, :], ysum[:])
        for q in (32, 64, 96):
            nc.sync.dma_start(R[q : q + 5, :], R[0:5, :])

        # ---- build L = [2x0;2x1;2x2; -|x|^2; -1] for fine and coarse ----
        def build_L(src_d, n):
            # src_d is [3, n] coordinate-major (host pre-transposed)
            Lt = lr.tile([128, n], FP32, tag=f"L{n}")
            nc.sync.dma_start(Lt[0:3, :], src_d[:, :])
            nc.sync.dma_start(Lt[4:5, :], consts_d[1:2, 0:n])
            C = n // 128
            xb = tmpp.tile([128, 3 * C], FP32, tag="yb")
            # [p, k, c] = coord k of point p*C+c  (p-major point blocks)
            nc.sync.dma_start(xb[:], src_d.rearrange("k (p c) -> p k c", p=128))
            nc.scalar.activation(xb[:], xb[:], mybir.ActivationFunctionType.Square)
            xsum = tmpp.tile([128, C], FP32, tag="ysum")
            xbv = xb.rearrange("p (k c) -> p k c", k=3)
            nc.vector.tensor_tensor(xsum[:], xbv[:, 0, :], xbv[:, 1, :], op=OP.add)
            nc.vector.tensor_tensor(xsum[:], xsum[:], xbv[:, 2, :], op=OP.add)
            nc.scalar.mul(xsum[:], xsum[:], -1.0)
            nc.sync.dma_start(Lt[3:4, :], xsum[:])
            for q in (32, 64, 96):
                nc.sync.dma_start(Lt[q : q + 5, :], Lt[0:5, :])
            return Lt

        Lf = build_L(fine_d, Nf)
        Lc = build_L(coarse_d, Ncs)

        # ---- accumulators ----
        colacc_f = accp.tile([128, Ng], FP16)
        colacc_c = accp.tile([128, Ng], FP16)
        nc.vector.memset(colacc_f[:], NEGF16)
        nc.vector.memset(colacc_c[:], NEGF16)
        rmins_f = accp.tile([128, Nf // 128], FP32)
        rmins_c = accp.tile([128, Ncs // 128], FP32)

        # ---- main pairwise pass ----
        with tc.tile_pool(name="psum", bufs=2, space="PSUM") as psum_pool:

            def cloud_pass(Lt, n_xt, colacc, rmins):
                for mi in range(n_xt):
                    scr_x = scrp.tile([128, Ng], FP16, tag="scrx")
                    for g in range(n_groups):
                        ps = psum_pool.tile([128, group], FP32, tag="ps")
                        for j in range(group // mm_n):
                            ny0 = g * group + j * mm_n
                            q = 32 * j
                            nc.tensor.matmul(
                                ps[:, j * mm_n : (j + 1) * mm_n],
                                Lt[q : q + 5, mi * 128 : (mi + 1) * 128],
                                R[q : q + 5, ny0 : ny0 + mm_n],
                                start=True,
                                stop=True,
                                tile_position=(q, 0),
                            )
                        # ScalarE drains PSUM to fp16 SBUF; DVE runs in fast
                        # 16-bit SBUF modes only.
                        gs = slice(g * group, (g + 1) * group)
                        nc.scalar.copy(scr_x[:, gs], ps[:])
                        nc.vector.tensor_tensor(
                            colacc[:, gs], colacc[:, gs], scr_x[:, gs], op=OP.max
                        )
                    # row direction: in-place halving fold tree (2x fp16 TT),
                    # then one 1x reduce of the last 512 columns.
                    w = Ng
                    while w > 512:
                        w //= 2
                        nc.vector.tensor_tensor(
                            scr_x[:, 0:w],
                            scr_x[:, 0:w],
                            scr_x[:, w : 2 * w],
                            op=OP.max,
                        )
                    nc.vector.reduce_max(
                        rmins[:, mi : mi + 1], scr_x[:, 0:w], axis=AX.X
                    )

            cloud_pass(Lf, Nf // 128, colacc_f, rmins_f)
            cloud_pass(Lc, Ncs // 128, colacc_c, rmins_c)

        # ---- column direction: partition-reduce colacc via PE transposes ----
        cmaxs_f = accp.tile([128, Ng // 128], FP32)
        cmaxs_c = accp.tile([128, Ng // 128], FP32)
        with tc.tile_pool(name="tpsum", bufs=4, space="PSUM") as tpsum:
            for colacc, cmaxs in ((colacc_f, cmaxs_f), (colacc_c, cmaxs_c)):
                for k in range(Ng // 128):
                    tp = tpsum.tile([128, 128], FP16, tag="tp")
                    nc.tensor.transpose(
                        tp[:], colacc[:, k * 128 : (k + 1) * 128], ident[:]
                    )
                    nc.vector.reduce_max(cmaxs[:, k : k + 1], tp[:], axis=AX.X)

        # ---- total sums -> 4 scalars ----
        # free-axis sums into a [128,4] matrix, then one K=128 matmul with a
        # ones vector does all four partition-axis sums at once.
        T4 = smallp.tile([128, 4], FP32)
        for idx, mat in enumerate((rmins_f, cmaxs_f, rmins_c, cmaxs_c)):
            nc.vector.reduce_sum(T4[:, idx : idx + 1], mat[:], axis=AX.X)
        ones_col = const.tile([128, 1], FP32)
        nc.vector.memset(ones_col[:], 1.0)
        with tc.tile_pool(name="fpsum", bufs=1, space="PSUM") as fpsum:
            outp = fpsum.tile([1, 4], FP32)
            nc.tensor.matmul(outp[:], ones_col[:], T4[:], start=True, stop=True)
            nc.vector.tensor_copy(out_sb[:], outp[:])

        nc.sync.dma_start(out_d[:, :], out_sb[:])

    return nc


def _register_ntff_hook():
    """The agent image's antenv lacks axon_hooks; synthesize the module and
    register the ctypes NTFF hook from trn_agent_boot so trace=True works."""
    import types

    try:
        from antenv import axon_hooks  # noqa: F401

        return True
    except ImportError:
        pass
    try:
        import antenv

        sys.path.insert(0, "/root/.axon_site")
        from trn_agent_boot.trn_boot import _ntff_profile_via_ctypes

        hook = _ntff_profile_via_ctypes("/opt/axon/libaxon_pjrt.so")
        mod = types.ModuleType("antenv.axon_hooks")
        _state = {"hook": hook}
        mod.set_axon_ntff_profile_hook = lambda h: _state.__setitem__("hook", h)
        mod.get_axon_ntff_profile_hook = lambda: _state["hook"]
        sys.modules["antenv.axon_hooks"] = mod
        antenv.axon_hooks = mod
        return hook is not None
    except Exception as e:  # profiling is best-effort
        print(f"ntff hook registration failed: {e}", file=sys.stderr)
        return False


_COMPILED = None


def _get_program():
    global _COMPILED
    if _COMPILED is None:
        nc = bass.Bass()
        emit_chamfer(nc, NF, NG, NC)
        cap_sync_waits(nc)
        _COMPILED = nc
    return _COMPILED


def kernel(coarse, fine, gt, alpha):
    global LAST_EXEC_NS
    from concourse.bass_utils import run_bass_kernel_spmd

    coarse = np.asarray(coarse, dtype=np.float32)
    fine = np.asarray(fine, dtype=np.float32)
    gt = np.asarray(gt, dtype=np.float32)
    alpha = np.asarray(alpha, dtype=np.float32)

    ident = np.eye(128, dtype=np.float16)
    consts = np.empty((2, max(NF, NG)), dtype=np.float32)
    consts[0] = 1.0
    consts[1] = -1.0
    in_maps = [
        {
            "fineT": np.ascontiguousarray(fine[b].T),
            "gt": np.ascontiguousarray(gt[b]),
            "coarseT": np.ascontiguousarray(coarse[b].T),
            "ident": ident,
            "consts": consts,
        }
        for b in range(B)
    ]

    nc = _get_program()
    trace = bool(int(os.environ.get("CHAMFER_TRACE", "0")))
    if trace:
        trace = _register_ntff_hook()
    res = run_bass_kernel_spmd(nc, in_maps, list(range(B)), trace=trace)
    if trace:
        LAST_EXEC_NS = res.exec_time_ns

    loss_fine_b = np.empty(B, dtype=np.float64)
    loss_coarse_b = np.empty(B, dtype=np.float64)
    for b in range(B):
        s = res.results[b]["out"].astype(np.float64).ravel()
        # s = [sum rowmax(-d) fine, sum colmax(-d) fine,
        #      sum rowmax(-d) coarse, sum colmax(-d) coarse]
        loss_fine_b[b] = -(s[0] / NF + s[1] / NG)
        loss_coarse_b[b] = -(s[2] / NC + s[3] / NG)

    loss_fine = loss_fine_b.mean()
    loss_coarse = loss_coarse_b.mean()
    loss = loss_coarse + float(alpha[0]) * loss_fine
    return (
        np.float32(loss),
        np.float32(loss_coarse),
        np.float32(loss_fine),
    )


# revision 32
# speedup vs baseline: 1.0005x; 1.0005x over previous
"""Chamfer-loss Trainium2 kernel.

kernel(coarse, fine, gt, alpha) -> (loss, loss_coarse, loss_fine)

Data-parallel over batch (B=8) across the 8 NeuronCores; each core computes
the two directed chamfer sums for fine<->gt and coarse<->gt of its batch
element by brute-force pairwise squared distances:

  -d(x,y) = 2*x.y - |x|^2 - |y|^2 is produced directly in PSUM by a K=5
  matmul: lhsT = [x0;x1;x2;-|x|^2;-1] (stationary [5,128] per x-tile),
  rhs = [2*y0;2*y1;2*y2;1;|y|^2] ([5,512] slices); "min distance" becomes
  "max of -d". The K=5 rows are replicated at partition bases 0/32/64/96 so
  FOUR row-group-packed matmuls (tile_position) run concurrently in the PE
  array (~4x PE throughput; K=5 uses only 5 of its 128 rows).

  ScalarE drains each [128,2048] PSUM group to fp16 SBUF. VectorE then runs
  entirely in fast 16-bit 2x mode: a tensor_tensor max into the column
  accumulator [128, Ng] (per-gt-point direction), and an in-place halving
  fold tree + one small reduce for the row direction (per-x-point min).
  Column accumulators are partition-reduced at the end via PE transposes +
  free-axis reduces, and all four totals are summed across partitions with
  a single K=128 ones-vector matmul.

  This walrus build accepts only one sync wait per instruction, so
  cap_sync_waits() post-processes the lowered program (see its docstring).

Host side only shards/transposes inputs, averages the per-core sums and
applies alpha. Measured: ~792us HW exec, rel err ~3e-6 vs the fp32
reference (single fp32->fp16 rounding on the distance values; mins/means
otherwise exact).
"""

import os
import sys
import numpy as np

sys.path.insert(0, "/opt/trn_rl_repo")

from contextlib import ExitStack

import concourse.bass as bass
import concourse.tile as tile
from concourse import mybir

FP32 = mybir.dt.float32
FP16 = mybir.dt.float16
AX = mybir.AxisListType
OP = mybir.AluOpType

NEG = -1.0e30
NEGF16 = -60000.0

# full-problem shapes (hardcoded; kernel.py must be self-contained)
B, NC, NF, NG = 8, 1024, 8192, 8192

LAST_EXEC_NS = None  # stashed HW exec time from the most recent traced run


def cap_sync_waits(nc):
    """This walrus build accepts only ONE sync wait per instruction.

    Compute-engine instructions: move overflow waits onto injected
    same-engine NoOps (sequencer FIFO preserves ordering).
    DMA instructions (separate queue processors -- a sequencer NoOp does
    NOT gate them): move ALL waits onto a Pool-engine NoOp chain whose
    last link increments a fresh auxiliary semaphore; the DMA then waits
    only on that semaphore.
    """
    used = set()
    for bb in nc.main_func.blocks:
        for ins in bb.instructions:
            si = ins.sync_info
            if si is not None:
                for w in si.on_wait or []:
                    used.add(w.id)
                for u in si.on_update or []:
                    used.add(u.id)
    aux = None
    for i in range(64):
        h = nc.alloc_semaphore(f"capw_aux{i}")
        if h.num not in used:
            aux = h
            break
    assert aux is not None, "no free semaphore for cap_sync_waits"
    aux_count = 0
    n_new = 0
    nid = [0]

    def mknop(engine, wait, update=None):
        nid[0] += 1
        nop = mybir.InstNoOp(name=f"capw-{nid[0]}", ins=[], outs=[])
        nop.engine = engine
        nop.sync_info = mybir.SyncInfo(
            on_wait=[wait] if wait is not None else [],
            on_update=[update] if update is not None else [],
        )
        nc.register_instruction(nop, overwrite=True)
        return nop

    for bb in nc.main_func.blocks:
        out = []
        changed = False
        for ins in bb.instructions:
            si = ins.sync_info
            waits = list(si.on_wait) if (si is not None and si.on_wait) else []
            if len(waits) > 1:
                changed = True
                is_dma = getattr(ins, "queue", None) is not None
                if is_dma:
                    aux_count += 1
                    for i, w in enumerate(waits):
                        upd = (
                            mybir.SyncUpdate(
                                sync_type="semaphore",
                                id=aux.num,
                                ant_name="capw_aux",
                                update_mode="sem-inc",
                                update_value=1,
                                update_reg=None,
                            )
                            if i == len(waits) - 1
                            else None
                        )
                        out.append(mknop(mybir.EngineType.Pool, w, upd))
                        n_new += 1
                    si.on_wait = [
                        mybir.SyncWait(
                            sync_type="semaphore",
                            id=aux.num,
                            ant_name="capw_aux",
                            wait_mode="sem-ge-imm",
                            wait_value=aux_count,
                            wait_reg=None,
                        )
                    ]
                else:
                    for w in waits[:-1]:
                        out.append(mknop(ins.engine, w))
                        n_new += 1
                    si.on_wait = waits[-1:]
            out.append(ins)
        if changed:
            bb.instructions = out
    return n_new


def emit_chamfer(nc, Nf, Ng, Ncs, group=2048):
    """Emit the full per-core program. Dims must divide (128, group)."""
    assert Ng % group == 0 and Nf % 128 == 0 and Ncs % 128 == 0
    n_groups = Ng // group
    mm_n = 512
    assert group % mm_n == 0

    fine_d = nc.dram_tensor("fineT", [3, Nf], FP32, kind="ExternalInput")
    gt_d = nc.dram_tensor("gt", [3, Ng], FP32, kind="ExternalInput")
    coarse_d = nc.dram_tensor("coarseT", [3, Ncs], FP32, kind="ExternalInput")
    ident_d = nc.dram_tensor("ident", [128, 128], FP16, kind="ExternalInput")
    # consts[0] = +1.0 row, consts[1] = -1.0 row (engine ops can't write
    # partitions 3/4 directly: partition base must be 0/32/64/96)
    consts_d = nc.dram_tensor("consts", [2, max(Nf, Ng)], FP32, kind="ExternalInput")
    out_d = nc.dram_tensor("out", [1, 4], FP32, kind="ExternalOutput")

    with ExitStack() as ctx:
        tc = ctx.enter_context(tile.TileContext(nc))
        const = ctx.enter_context(tc.tile_pool(name="const", bufs=1))
        lr = ctx.enter_context(tc.tile_pool(name="lr", bufs=1))
        accp = ctx.enter_context(tc.tile_pool(name="accp", bufs=1))
        tmpp = ctx.enter_context(tc.tile_pool(name="tmpp", bufs=3))
        scrp = ctx.enter_context(tc.tile_pool(name="scrp", bufs=3))
        smallp = ctx.enter_context(tc.tile_pool(name="smallp", bufs=4))

        ident = const.tile([128, 128], FP16)
        nc.sync.dma_start(ident[:], ident_d[:, :])
        out_sb = const.tile([1, 4], FP32)

        # ---- build R = [y0;y1;y2; 1; |y|^2]  [5, Ng] ----
        # |y|^2 computed in a [128, Ng/128] block layout ([p,c] = y_{c*128+p}),
        # then DMA'd into partition 4 (engine ops can't start at partition 4).
        # R/L are allocated [128, N]: the K=5 operand rows are replicated at
        # base partitions 0/32/64/96 so four row-group-packed matmuls can run
        # concurrently in the PE array (K=5 uses only 5 of 128 rows).
        R = lr.tile([128, Ng], FP32)
        nc.sync.dma_start(R[0:3, :], gt_d[:, :])
        nc.scalar.mul(R[0:3, :], R[0:3, :], 2.0)
        nc.sync.dma_start(R[3:4, :], consts_d[0:1, 0:Ng])
        Cg = Ng // 128
        yb = tmpp.tile([128, 3 * Cg], FP32, tag="yb")
        # [p, k, c] = y-coord k of point p*Cg+c  (p-major point blocks)
        nc.sync.dma_start(yb[:], gt_d.rearrange("k (p c) -> p k c", p=128))
        nc.scalar.activation(yb[:], yb[:], mybir.ActivationFunctionType.Square)
        ysum = tmpp.tile([128, Cg], FP32, tag="ysum")
        ybv = yb.rearrange("p (k c) -> p k c", k=3)
        nc.vector.tensor_tensor(ysum[:], ybv[:, 0, :], ybv[:, 1, :], op=OP.add)
        nc.vector.tensor_tensor(ysum[:], ysum[:], ybv[:, 2, :], op=OP.add)
        nc.sync.dma_start(R[4:5, :], ysum[:])
        for q in (32, 64, 96):
            nc.sync.dma_start(R[q : q + 5, :], R[0:5, :])

        # ---- build L = [2x0;2x1;2x2; -|x|^2; -1] for fine and coarse ----
        def build_L(src_d, n):
            # src_d is [3, n] coordinate-major (host pre-transposed)
            Lt = lr.tile([128, n], FP32, tag=f"L{n}")
            nc.sync.dma_start(Lt[0:3, :], src_d[:, :])
            nc.sync.dma_start(Lt[4:5, :], consts_d[1:2, 0:n])
            C = n // 128
            xb = tmpp.tile([128, 3 * C], FP32, tag="yb")
            # [p, k, c] = coord k of point p*C+c  (p-major point blocks)
            nc.sync.dma_start(xb[:], src_d.rearrange("k (p c) -> p k c", p=128))
            nc.scalar.activation(xb[:], xb[:], mybir.ActivationFunctionType.Square)
            xsum = tmpp.tile([128, C], FP32, tag="ysum")
            xbv = xb.rearrange("p (k c) -> p k c", k=3)
            nc.vector.tensor_tensor(xsum[:], xbv[:, 0, :], xbv[:, 1, :], op=OP.add)
            nc.vector.tensor_tensor(xsum[:], xsum[:], xbv[:, 2, :], op=OP.add)
            nc.scalar.mul(xsum[:], xsum[:], -1.0)
            nc.sync.dma_start(Lt[3:4, :], xsum[:])
            for q in (32, 64, 96):
                nc.sync.dma_start(Lt[q : q + 5, :], Lt[0:5, :])
            return Lt

        Lf = build_L(fine_d, Nf)
        Lc = build_L(coarse_d, Ncs)

        # ---- accumulators ----
        colacc_f = accp.tile([128, Ng], FP16)
        colacc_c = accp.tile([128, Ng], FP16)
        nc.vector.memset(colacc_f[:], NEGF16)
        nc.vector.memset(colacc_c[:], NEGF16)
        rmins_f = accp.tile([128, Nf // 128], FP32)
        rmins_c = accp.tile([128, Ncs // 128], FP32)

        # ---- main pairwise pass ----
        with tc.tile_pool(name="psum", bufs=2, space="PSUM") as psum_pool:

            def cloud_pass(Lt, n_xt, colacc, rmins):
                for mi in range(n_xt):
                    scr_x = scrp.tile([128, Ng], FP16, tag="scrx")
                    for g in range(n_groups):
                        ps = psum_pool.tile([128, group], FP32, tag="ps")
                        for j in range(group // mm_n):
                            ny0 = g * group + j * mm_n
                            q = 32 * j
                            nc.tensor.matmul(
                                ps[:, j * mm_n : (j + 1) * mm_n],
                                Lt[q : q + 5, mi * 128 : (mi + 1) * 128],
                                R[q : q + 5, ny0 : ny0 + mm_n],
                                start=True,
                                stop=True,
                                tile_position=(q, 0),
                            )
                        # ScalarE drains PSUM to fp16 SBUF; DVE runs in fast
                        # 16-bit SBUF modes only.
                        gs = slice(g * group, (g + 1) * group)
                        nc.scalar.copy(scr_x[:, gs], ps[:])
                        nc.vector.tensor_tensor(
                            colacc[:, gs], colacc[:, gs], scr_x[:, gs], op=OP.max
                        )
                    # row direction: in-place halving fold tree (2x fp16 TT),
                    # then one 1x reduce of the last 512 columns.
                    w = Ng
                    while w > 512:
                        w //= 2
                        nc.vector.tensor_tensor(
                            scr_x[:, 0:w],
                            scr_x[:, 0:w],
                            scr_x[:, w : 2 * w],
                            op=OP.max,
                        )
                    nc.vector.reduce_max(
                        rmins[:, mi : mi + 1], scr_x[:, 0:w], axis=AX.X
                    )

            cloud_pass(Lf, Nf // 128, colacc_f, rmins_f)
            cloud_pass(Lc, Ncs // 128, colacc_c, rmins_c)

        # ---- column direction: partition-reduce colacc via PE transposes ----
        cmaxs_f = accp.tile([128, Ng // 128], FP32)
        cmaxs_c = accp.tile([128, Ng // 128], FP32)
        with tc.tile_pool(name="tpsum", bufs=4, space="PSUM") as tpsum:
            for colacc, cmaxs in ((colacc_f, cmaxs_f), (colacc_c, cmaxs_c)):
                for k in range(Ng // 128):
                    tp = tpsum.tile([128, 128], FP16, tag="tp")
                    nc.tensor.transpose(
                        tp[:], colacc[:, k * 128 : (k + 1) * 128], ident[:]
                    )
                    nc.vector.reduce_max(cmaxs[:, k : k + 1], tp[:], axis=AX.X)

        # ---- total sums -> 4 scalars ----
        # free-axis sums into a [128,4] matrix, then one K=128 matmul with a
        # ones vector does all four partition-axis sums at once.
        T4 = smallp.tile([128, 4], FP32)
        for idx, mat in enumerate((rmins_f, cmaxs_f, rmins_c, cmaxs_c)):
            nc.vector.reduce_sum(T4[:, idx : idx + 1], mat[:], axis=AX.X)
        ones_col = const.tile([128, 1], FP32)
        nc.vector.memset(ones_col[:], 1.0)
        with tc.tile_pool(name="fpsum", bufs=1, space="PSUM") as fpsum:
            outp = fpsum.tile([1, 4], FP32)
            nc.tensor.matmul(outp[:], ones_col[:], T4[:], start=True, stop=True)
            nc.vector.tensor_copy(out_sb[:], outp[:])

        nc.sync.dma_start(out_d[:, :], out_sb[:])

    return nc


def _register_ntff_hook():
    """The agent image's antenv lacks axon_hooks; synthesize the module and
    register the ctypes NTFF hook from trn_agent_boot so trace=True works."""
    import types

    try:
        from antenv import axon_hooks  # noqa: F401

        return True
    except ImportError:
        pass
    try:
        import antenv

        sys.path.insert(0, "/root/.axon_site")
        from trn_agent_boot.trn_boot import _ntff_profile_via_ctypes

        hook = _ntff_profile_via_ctypes("/opt/axon/libaxon_pjrt.so")
        mod = types.ModuleType("antenv.axon_hooks")
        _state = {"hook": hook}
        mod.set_axon_ntff_profile_hook = lambda h: _state.__setitem__("hook", h)
        mod.get_axon_ntff_profile_hook = lambda: _state["hook"]
        sys.modules["antenv.axon_hooks"] = mod
        antenv.axon_hooks = mod
        return hook is not None
    except Exception as e:  # profiling is best-effort
        print(f"ntff hook registration failed: {e}", file=sys.stderr)
        return False


_COMPILED = None


def _get_program():
    global _COMPILED
    if _COMPILED is None:
        nc = bass.Bass()
        emit_chamfer(nc, NF, NG, NC)
        cap_sync_waits(nc)
        _COMPILED = nc
    return _COMPILED


def kernel(coarse, fine, gt, alpha):
    global LAST_EXEC_NS
    from concourse.bass_utils import run_bass_kernel_spmd

    coarse = np.asarray(coarse, dtype=np.float32)
    fine = np.asarray(fine, dtype=np.float32)
    gt = np.asarray(gt, dtype=np.float32)
    alpha = np.asarray(alpha, dtype=np.float32)

    ident = np.eye(128, dtype=np.float16)
    consts = np.empty((2, max(NF, NG)), dtype=np.float32)
    consts[0] = 1.0
    consts[1] = -1.0
    in_maps = [
        {
            "fineT": np.ascontiguousarray(fine[b].T),
            "gt": np.ascontiguousarray(gt[b]),
            "coarseT": np.ascontiguousarray(coarse[b].T),
            "ident": ident,
            "consts": consts,
        }
        for b in range(B)
    ]

    nc = _get_program()
    trace = bool(int(os.environ.get("CHAMFER_TRACE", "0")))
    if trace:
        trace = _register_ntff_hook()
    res = run_bass_kernel_spmd(nc, in_maps, list(range(B)), trace=trace)
    if trace:
        LAST_EXEC_NS = res.exec_time_ns

    loss_fine_b = np.empty(B, dtype=np.float64)
    loss_coarse_b = np.empty(B, dtype=np.float64)
    for b in range(B):
        s = res.results[b]["out"].astype(np.float64).ravel()
        # s = [sum rowmax(-d) fine, sum colmax(-d) fine,
        #      sum rowmax(-d) coarse, sum colmax(-d) coarse]
        loss_fine_b[b] = -(s[0] / NF + s[1] / NG)
        loss_coarse_b[b] = -(s[2] / NC + s[3] / NG)

    loss_fine = loss_fine_b.mean()
    loss_coarse = loss_coarse_b.mean()
    loss = loss_coarse + float(alpha[0]) * loss_fine
    return (
        np.float32(loss),
        np.float32(loss_coarse),
        np.float32(loss_fine),
    )


# revision 33
# speedup vs baseline: 1.0054x; 1.0050x over previous
"""Chamfer-loss Trainium2 kernel.

kernel(coarse, fine, gt, alpha) -> (loss, loss_coarse, loss_fine)

Data-parallel over batch (B=8) across the 8 NeuronCores; each core computes
the two directed chamfer sums for fine<->gt and coarse<->gt of its batch
element by brute-force pairwise squared distances:

  -d(x,y) = 2*x.y - |x|^2 - |y|^2 is produced directly in PSUM by a K=5
  matmul: lhsT = [x0;x1;x2;-|x|^2;-1] (stationary [5,128] per x-tile),
  rhs = [2*y0;2*y1;2*y2;1;|y|^2] ([5,512] slices); "min distance" becomes
  "max of -d". The K=5 rows are replicated at partition bases 0/32/64/96 so
  FOUR row-group-packed matmuls (tile_position) run concurrently in the PE
  array (~4x PE throughput; K=5 uses only 5 of its 128 rows).

  ScalarE drains each [128,2048] PSUM group to fp16 SBUF. VectorE then runs
  entirely in fast 16-bit 2x mode: a tensor_tensor max into the column
  accumulator [128, Ng] (per-gt-point direction), and an in-place halving
  fold tree + one small reduce for the row direction (per-x-point min).
  Column accumulators are partition-reduced at the end via PE transposes +
  free-axis reduces, and all four totals are summed across partitions with
  a single K=128 ones-vector matmul.

  This walrus build accepts only one sync wait per instruction, so
  cap_sync_waits() post-processes the lowered program (see its docstring).

Host side only shards/transposes inputs, averages the per-core sums and
applies alpha. Measured: ~792us HW exec, rel err ~3e-6 vs the fp32
reference (single fp32->fp16 rounding on the distance values; mins/means
otherwise exact).
"""

import os
import sys
import numpy as np

sys.path.insert(0, "/opt/trn_rl_repo")

from contextlib import ExitStack

import concourse.bass as bass
import concourse.tile as tile
from concourse import mybir

FP32 = mybir.dt.float32
FP16 = mybir.dt.float16
AX = mybir.AxisListType
OP = mybir.AluOpType

NEG = -1.0e30
NEGF16 = -60000.0

# full-problem shapes (hardcoded; kernel.py must be self-contained)
B, NC, NF, NG = 8, 1024, 8192, 8192

LAST_EXEC_NS = None  # stashed HW exec time from the most recent traced run


def cap_sync_waits(nc):
    """This walrus build accepts only ONE sync wait per instruction.

    Compute-engine instructions: move overflow waits onto injected
    same-engine NoOps (sequencer FIFO preserves ordering).
    DMA instructions (separate queue processors -- a sequencer NoOp does
    NOT gate them): move ALL waits onto a Pool-engine NoOp chain whose
    last link increments a fresh auxiliary semaphore; the DMA then waits
    only on that semaphore.
    """
    used = set()
    for bb in nc.main_func.blocks:
        for ins in bb.instructions:
            si = ins.sync_info
            if si is not None:
                for w in si.on_wait or []:
                    used.add(w.id)
                for u in si.on_update or []:
                    used.add(u.id)
    aux = None
    for i in range(64):
        h = nc.alloc_semaphore(f"capw_aux{i}")
        if h.num not in used:
            aux = h
            break
    assert aux is not None, "no free semaphore for cap_sync_waits"
    aux_count = 0
    n_new = 0
    nid = [0]

    def mknop(engine, wait, update=None):
        nid[0] += 1
        nop = mybir.InstNoOp(name=f"capw-{nid[0]}", ins=[], outs=[])
        nop.engine = engine
        nop.sync_info = mybir.SyncInfo(
            on_wait=[wait] if wait is not None else [],
            on_update=[update] if update is not None else [],
        )
        nc.register_instruction(nop, overwrite=True)
        return nop

    for bb in nc.main_func.blocks:
        out = []
        changed = False
        for ins in bb.instructions:
            si = ins.sync_info
            waits = list(si.on_wait) if (si is not None and si.on_wait) else []
            if len(waits) > 1:
                changed = True
                is_dma = getattr(ins, "queue", None) is not None
                if is_dma:
                    aux_count += 1
                    for i, w in enumerate(waits):
                        upd = (
                            mybir.SyncUpdate(
                                sync_type="semaphore",
                                id=aux.num,
                                ant_name="capw_aux",
                                update_mode="sem-inc",
                                update_value=1,
                                update_reg=None,
                            )
                            if i == len(waits) - 1
                            else None
                        )
                        out.append(mknop(mybir.EngineType.Pool, w, upd))
                        n_new += 1
                    si.on_wait = [
                        mybir.SyncWait(
                            sync_type="semaphore",
                            id=aux.num,
                            ant_name="capw_aux",
                            wait_mode="sem-ge-imm",
                            wait_value=aux_count,
                            wait_reg=None,
                        )
                    ]
                else:
                    for w in waits[:-1]:
                        out.append(mknop(ins.engine, w))
                        n_new += 1
                    si.on_wait = waits[-1:]
            out.append(ins)
        if changed:
            bb.instructions = out
    return n_new


def emit_chamfer(nc, Nf, Ng, Ncs, group=2048):
    """Emit the full per-core program. Dims must divide (128, group)."""
    assert Ng % group == 0 and Nf % 128 == 0 and Ncs % 128 == 0
    n_groups = Ng // group
    mm_n = 512
    assert group % mm_n == 0

    fine_d = nc.dram_tensor("fineT", [3, Nf], FP32, kind="ExternalInput")
    gt_d = nc.dram_tensor("gt", [3, Ng], FP32, kind="ExternalInput")
    coarse_d = nc.dram_tensor("coarseT", [3, Ncs], FP32, kind="ExternalInput")
    ident_d = nc.dram_tensor("ident", [128, 128], FP16, kind="ExternalInput")
    # consts[0] = +1.0 row, consts[1] = -1.0 row (engine ops can't write
    # partitions 3/4 directly: partition base must be 0/32/64/96)
    consts_d = nc.dram_tensor("consts", [2, max(Nf, Ng)], FP32, kind="ExternalInput")
    out_d = nc.dram_tensor("out", [1, 4], FP32, kind="ExternalOutput")

    with ExitStack() as ctx:
        tc = ctx.enter_context(tile.TileContext(nc))
        const = ctx.enter_context(tc.tile_pool(name="const", bufs=1))
        lr = ctx.enter_context(tc.tile_pool(name="lr", bufs=1))
        accp = ctx.enter_context(tc.tile_pool(name="accp", bufs=1))
        tmpp = ctx.enter_context(tc.tile_pool(name="tmpp", bufs=3))
        scrp = ctx.enter_context(tc.tile_pool(name="scrp", bufs=4))
        smallp = ctx.enter_context(tc.tile_pool(name="smallp", bufs=4))

        ident = const.tile([128, 128], FP16)
        nc.sync.dma_start(ident[:], ident_d[:, :])
        out_sb = const.tile([1, 4], FP32)

        # ---- build R = [y0;y1;y2; 1; |y|^2]  [5, Ng] ----
        # |y|^2 computed in a [128, Ng/128] block layout ([p,c] = y_{c*128+p}),
        # then DMA'd into partition 4 (engine ops can't start at partition 4).
        # R/L are allocated [128, N]: the K=5 operand rows are replicated at
        # base partitions 0/32/64/96 so four row-group-packed matmuls can run
        # concurrently in the PE array (K=5 uses only 5 of 128 rows).
        R = lr.tile([128, Ng], FP32)
        nc.sync.dma_start(R[0:3, :], gt_d[:, :])
        nc.scalar.mul(R[0:3, :], R[0:3, :], 2.0)
        nc.sync.dma_start(R[3:4, :], consts_d[0:1, 0:Ng])
        Cg = Ng // 128
        yb = tmpp.tile([128, 3 * Cg], FP32, tag="yb")
        # [p, k, c] = y-coord k of point p*Cg+c  (p-major point blocks)
        nc.sync.dma_start(yb[:], gt_d.rearrange("k (p c) -> p k c", p=128))
        nc.scalar.activation(yb[:], yb[:], mybir.ActivationFunctionType.Square)
        ysum = tmpp.tile([128, Cg], FP32, tag="ysum")
        ybv = yb.rearrange("p (k c) -> p k c", k=3)
        nc.vector.tensor_tensor(ysum[:], ybv[:, 0, :], ybv[:, 1, :], op=OP.add)
        nc.vector.tensor_tensor(ysum[:], ysum[:], ybv[:, 2, :], op=OP.add)
        nc.sync.dma_start(R[4:5, :], ysum[:])
        for q in (32, 64, 96):
            nc.sync.dma_start(R[q : q + 5, :], R[0:5, :])

        # ---- build L = [2x0;2x1;2x2; -|x|^2; -1] for fine and coarse ----
        def build_L(src_d, n):
            # src_d is [3, n] coordinate-major (host pre-transposed)
            Lt = lr.tile([128, n], FP32, tag=f"L{n}")
            nc.sync.dma_start(Lt[0:3, :], src_d[:, :])
            nc.sync.dma_start(Lt[4:5, :], consts_d[1:2, 0:n])
            C = n // 128
            xb = tmpp.tile([128, 3 * C], FP32, tag="yb")
            # [p, k, c] = coord k of point p*C+c  (p-major point blocks)
            nc.sync.dma_start(xb[:], src_d.rearrange("k (p c) -> p k c", p=128))
            nc.scalar.activation(xb[:], xb[:], mybir.ActivationFunctionType.Square)
            xsum = tmpp.tile([128, C], FP32, tag="ysum")
            xbv = xb.rearrange("p (k c) -> p k c", k=3)
            nc.vector.tensor_tensor(xsum[:], xbv[:, 0, :], xbv[:, 1, :], op=OP.add)
            nc.vector.tensor_tensor(xsum[:], xsum[:], xbv[:, 2, :], op=OP.add)
            nc.scalar.mul(xsum[:], xsum[:], -1.0)
            nc.sync.dma_start(Lt[3:4, :], xsum[:])
            for q in (32, 64, 96):
                nc.sync.dma_start(Lt[q : q + 5, :], Lt[0:5, :])
            return Lt

        Lf = build_L(fine_d, Nf)
        Lc = build_L(coarse_d, Ncs)

        # ---- accumulators ----
        colacc_f = accp.tile([128, Ng], FP16)
        colacc_c = accp.tile([128, Ng], FP16)
        rmins_f = accp.tile([128, Nf // 128], FP32)
        rmins_c = accp.tile([128, Ncs // 128], FP32)

        # ---- main pairwise pass ----
        with tc.tile_pool(name="psum", bufs=2, space="PSUM") as psum_pool:

            def cloud_pass(Lt, n_xt, colacc, rmins):
                for mi in range(n_xt):
                    scr_x = scrp.tile([128, Ng], FP16, tag="scrx")
                    for g in range(n_groups):
                        ps = psum_pool.tile([128, group], FP32, tag="ps")
                        for j in range(group // mm_n):
                            ny0 = g * group + j * mm_n
                            q = 32 * j
                            nc.tensor.matmul(
                                ps[:, j * mm_n : (j + 1) * mm_n],
                                Lt[q : q + 5, mi * 128 : (mi + 1) * 128],
                                R[q : q + 5, ny0 : ny0 + mm_n],
                                start=True,
                                stop=True,
                                tile_position=(q, 0),
                            )
                        # ScalarE drains PSUM to fp16 SBUF; DVE runs in fast
                        # 16-bit SBUF modes only.
                        gs = slice(g * group, (g + 1) * group)
                        nc.scalar.copy(scr_x[:, gs], ps[:])
                        if mi == 0:
                            # first x-tile initializes colacc (4x fp16 copy)
                            nc.vector.tensor_copy(colacc[:, gs], scr_x[:, gs])
                        else:
                            nc.vector.tensor_tensor(
                                colacc[:, gs], colacc[:, gs], scr_x[:, gs], op=OP.max
                            )
                    # row direction: in-place halving fold tree (2x fp16 TT),
                    # then one 1x reduce of the last 512 columns.
                    w = Ng
                    while w > 512:
                        w //= 2
                        nc.vector.tensor_tensor(
                            scr_x[:, 0:w],
                            scr_x[:, 0:w],
                            scr_x[:, w : 2 * w],
                            op=OP.max,
                        )
                    nc.vector.reduce_max(
                        rmins[:, mi : mi + 1], scr_x[:, 0:w], axis=AX.X
                    )

            cloud_pass(Lf, Nf // 128, colacc_f, rmins_f)
            cloud_pass(Lc, Ncs // 128, colacc_c, rmins_c)

        # ---- column direction: partition-reduce colacc via PE transposes ----
        cmaxs_f = accp.tile([128, Ng // 128], FP32)
        cmaxs_c = accp.tile([128, Ng // 128], FP32)
        with tc.tile_pool(name="tpsum", bufs=4, space="PSUM") as tpsum:
            for colacc, cmaxs in ((colacc_f, cmaxs_f), (colacc_c, cmaxs_c)):
                for k in range(Ng // 128):
                    tp = tpsum.tile([128, 128], FP16, tag="tp")
                    nc.tensor.transpose(
                        tp[:], colacc[:, k * 128 : (k + 1) * 128], ident[:]
                    )
                    nc.vector.reduce_max(cmaxs[:, k : k + 1], tp[:], axis=AX.X)

        # ---- total sums -> 4 scalars ----
        # free-axis sums into a [128,4] matrix, then one K=128 matmul with a
        # ones vector does all four partition-axis sums at once.
        T4 = smallp.tile([128, 4], FP32)
        for idx, mat in enumerate((rmins_f, cmaxs_f, rmins_c, cmaxs_c)):
            nc.vector.reduce_sum(T4[:, idx : idx + 1], mat[:], axis=AX.X)
        ones_col = const.tile([128, 1], FP32)
        nc.vector.memset(ones_col[:], 1.0)
        with tc.tile_pool(name="fpsum", bufs=1, space="PSUM") as fpsum:
            outp = fpsum.tile([1, 4], FP32)
            nc.tensor.matmul(outp[:], ones_col[:], T4[:], start=True, stop=True)
            nc.vector.tensor_copy(out_sb[:], outp[:])

        nc.sync.dma_start(out_d[:, :], out_sb[:])

    return nc


def _register_ntff_hook():
    """The agent image's antenv lacks axon_hooks; synthesize the module and
    register the ctypes NTFF hook from trn_agent_boot so trace=True works."""
    import types

    try:
        from antenv import axon_hooks  # noqa: F401

        return True
    except ImportError:
        pass
    try:
        import antenv

        sys.path.insert(0, "/root/.axon_site")
        from trn_agent_boot.trn_boot import _ntff_profile_via_ctypes

        hook = _ntff_profile_via_ctypes("/opt/axon/libaxon_pjrt.so")
        mod = types.ModuleType("antenv.axon_hooks")
        _state = {"hook": hook}
        mod.set_axon_ntff_profile_hook = lambda h: _state.__setitem__("hook", h)
        mod.get_axon_ntff_profile_hook = lambda: _state["hook"]
        sys.modules["antenv.axon_hooks"] = mod
        antenv.axon_hooks = mod
        return hook is not None
    except Exception as e:  # profiling is best-effort
        print(f"ntff hook registration failed: {e}", file=sys.stderr)
        return False


_COMPILED = None


def _get_program():
    global _COMPILED
    if _COMPILED is None:
        nc = bass.Bass()
        emit_chamfer(nc, NF, NG, NC)
        cap_sync_waits(nc)
        _COMPILED = nc
    return _COMPILED


def kernel(coarse, fine, gt, alpha):
    global LAST_EXEC_NS
    from concourse.bass_utils import run_bass_kernel_spmd

    coarse = np.asarray(coarse, dtype=np.float32)
    fine = np.asarray(fine, dtype=np.float32)
    gt = np.asarray(gt, dtype=np.float32)
    alpha = np.asarray(alpha, dtype=np.float32)

    ident = np.eye(128, dtype=np.float16)
    consts = np.empty((2, max(NF, NG)), dtype=np.float32)
    consts[0] = 1.0
    consts[1] = -1.0
    in_maps = [
        {
            "fineT": np.ascontiguousarray(fine[b].T),
            "gt": np.ascontiguousarray(gt[b]),
            "coarseT": np.ascontiguousarray(coarse[b].T),
            "ident": ident,
            "consts": consts,
        }
        for b in range(B)
    ]

    nc = _get_program()
    trace = bool(int(os.environ.get("CHAMFER_TRACE", "0")))
    if trace:
        trace = _register_ntff_hook()
    res = run_bass_kernel_spmd(nc, in_maps, list(range(B)), trace=trace)
    if trace:
        LAST_EXEC_NS = res.exec_time_ns

    loss_fine_b = np.empty(B, dtype=np.float64)
    loss_coarse_b = np.empty(B, dtype=np.float64)
    for b in range(B):
        s = res.results[b]["out"].astype(np.float64).ravel()
        # s = [sum rowmax(-d) fine, sum colmax(-d) fine,
        #      sum rowmax(-d) coarse, sum colmax(-d) coarse]
        loss_fine_b[b] = -(s[0] / NF + s[1] / NG)
        loss_coarse_b[b] = -(s[2] / NC + s[3] / NG)

    loss_fine = loss_fine_b.mean()
    loss_coarse = loss_coarse_b.mean()
    loss = loss_coarse + float(alpha[0]) * loss_fine
    return (
        np.float32(loss),
        np.float32(loss_coarse),
        np.float32(loss_fine),
    )


# revision 35
# speedup vs baseline: 1.0056x; 1.0002x over previous
"""Chamfer-loss Trainium2 kernel.

kernel(coarse, fine, gt, alpha) -> (loss, loss_coarse, loss_fine)

Data-parallel over batch (B=8) across the 8 NeuronCores; each core computes
the two directed chamfer sums for fine<->gt and coarse<->gt of its batch
element by brute-force pairwise squared distances:

  -d(x,y) = 2*x.y - |x|^2 - |y|^2 is produced directly in PSUM by a K=5
  matmul: lhsT = [x0;x1;x2;-|x|^2;-1] (stationary [5,128] per x-tile),
  rhs = [2*y0;2*y1;2*y2;1;|y|^2] ([5,512] slices); "min distance" becomes
  "max of -d". The K=5 rows are replicated at partition bases 0/32/64/96 so
  FOUR row-group-packed matmuls (tile_position) run concurrently in the PE
  array (~4x PE throughput; K=5 uses only 5 of its 128 rows).

  ScalarE drains each [128,2048] PSUM group to fp16 SBUF. VectorE then runs
  entirely in fast 16-bit 2x mode: a tensor_tensor max into the column
  accumulator [128, Ng] (per-gt-point direction), and an in-place halving
  fold tree + one small reduce for the row direction (per-x-point min).
  Column accumulators are partition-reduced at the end via PE transposes +
  free-axis reduces, and all four totals are summed across partitions with
  a single K=128 ones-vector matmul.

  This walrus build accepts only one sync wait per instruction, so
  cap_sync_waits() post-processes the lowered program (see its docstring).

Host side only shards/transposes inputs, averages the per-core sums and
applies alpha. Measured: ~792us HW exec, rel err ~3e-6 vs the fp32
reference (single fp32->fp16 rounding on the distance values; mins/means
otherwise exact).
"""

import os
import sys
import numpy as np

sys.path.insert(0, "/opt/trn_rl_repo")

from contextlib import ExitStack

import concourse.bass as bass
import concourse.tile as tile
from concourse import mybir

FP32 = mybir.dt.float32
FP16 = mybir.dt.float16
AX = mybir.AxisListType
OP = mybir.AluOpType

NEG = -1.0e30
NEGF16 = -60000.0

# full-problem shapes (hardcoded; kernel.py must be self-contained)
B, NC, NF, NG = 8, 1024, 8192, 8192

LAST_EXEC_NS = None  # stashed HW exec time from the most recent traced run


def cap_sync_waits(nc):
    """This walrus build accepts only ONE sync wait per instruction.

    Compute-engine instructions: move overflow waits onto injected
    same-engine NoOps (sequencer FIFO preserves ordering).
    DMA instructions (separate queue processors -- a sequencer NoOp does
    NOT gate them): move ALL waits onto a Pool-engine NoOp chain whose
    last link increments a fresh auxiliary semaphore; the DMA then waits
    only on that semaphore.
    """
    used = set()
    for bb in nc.main_func.blocks:
        for ins in bb.instructions:
            si = ins.sync_info
            if si is not None:
                for w in si.on_wait or []:
                    used.add(w.id)
                for u in si.on_update or []:
                    used.add(u.id)
    aux = None
    for i in range(64):
        h = nc.alloc_semaphore(f"capw_aux{i}")
        if h.num not in used:
            aux = h
            break
    assert aux is not None, "no free semaphore for cap_sync_waits"
    aux_count = 0
    n_new = 0
    nid = [0]

    def mknop(engine, wait, update=None):
        nid[0] += 1
        nop = mybir.InstNoOp(name=f"capw-{nid[0]}", ins=[], outs=[])
        nop.engine = engine
        nop.sync_info = mybir.SyncInfo(
            on_wait=[wait] if wait is not None else [],
            on_update=[update] if update is not None else [],
        )
        nc.register_instruction(nop, overwrite=True)
        return nop

    for bb in nc.main_func.blocks:
        out = []
        changed = False
        for ins in bb.instructions:
            si = ins.sync_info
            waits = list(si.on_wait) if (si is not None and si.on_wait) else []
            if len(waits) > 1:
                changed = True
                is_dma = getattr(ins, "queue", None) is not None
                if is_dma:
                    aux_count += 1
                    for i, w in enumerate(waits):
                        upd = (
                            mybir.SyncUpdate(
                                sync_type="semaphore",
                                id=aux.num,
                                ant_name="capw_aux",
                                update_mode="sem-inc",
                                update_value=1,
                                update_reg=None,
                            )
                            if i == len(waits) - 1
                            else None
                        )
                        out.append(mknop(mybir.EngineType.Pool, w, upd))
                        n_new += 1
                    si.on_wait = [
                        mybir.SyncWait(
                            sync_type="semaphore",
                            id=aux.num,
                            ant_name="capw_aux",
                            wait_mode="sem-ge-imm",
                            wait_value=aux_count,
                            wait_reg=None,
                        )
                    ]
                else:
                    for w in waits[:-1]:
                        out.append(mknop(ins.engine, w))
                        n_new += 1
                    si.on_wait = waits[-1:]
            out.append(ins)
        if changed:
            bb.instructions = out
    return n_new


def emit_chamfer(nc, Nf, Ng, Ncs, group=2048):
    """Emit the full per-core program. Dims must divide (128, group)."""
    assert Ng % group == 0 and Nf % 128 == 0 and Ncs % 128 == 0
    n_groups = Ng // group
    mm_n = 512
    assert group % mm_n == 0

    fine_d = nc.dram_tensor("fineT", [3, Nf], FP32, kind="ExternalInput")
    gt_d = nc.dram_tensor("gt", [3, Ng], FP32, kind="ExternalInput")
    coarse_d = nc.dram_tensor("coarseT", [3, Ncs], FP32, kind="ExternalInput")
    ident_d = nc.dram_tensor("ident", [128, 128], FP16, kind="ExternalInput")
    # consts[0] = +1.0 row, consts[1] = -1.0 row (engine ops can't write
    # partitions 3/4 directly: partition base must be 0/32/64/96)
    consts_d = nc.dram_tensor("consts", [2, max(Nf, Ng)], FP32, kind="ExternalInput")
    out_d = nc.dram_tensor("out", [1, 4], FP32, kind="ExternalOutput")

    with ExitStack() as ctx:
        tc = ctx.enter_context(tile.TileContext(nc))
        const = ctx.enter_context(tc.tile_pool(name="const", bufs=1))
        lr = ctx.enter_context(tc.tile_pool(name="lr", bufs=1))
        accp = ctx.enter_context(tc.tile_pool(name="accp", bufs=1))
        tmpp = ctx.enter_context(tc.tile_pool(name="tmpp", bufs=3))
        scrp = ctx.enter_context(tc.tile_pool(name="scrp", bufs=4))
        smallp = ctx.enter_context(tc.tile_pool(name="smallp", bufs=4))

        ident = const.tile([128, 128], FP16)
        nc.sync.dma_start(ident[:], ident_d[:, :])
        out_sb = const.tile([1, 4], FP32)

        # ---- build R = [y0;y1;y2; 1; |y|^2]  [5, Ng] ----
        # |y|^2 computed in a [128, Ng/128] block layout ([p,c] = y_{c*128+p}),
        # then DMA'd into partition 4 (engine ops can't start at partition 4).
        # R/L are allocated [128, N]: the K=5 operand rows are replicated at
        # base partitions 0/32/64/96 so four row-group-packed matmuls can run
        # concurrently in the PE array (K=5 uses only 5 of 128 rows).
        R = lr.tile([128, Ng], FP32)
        nc.sync.dma_start(R[0:3, :], gt_d[:, :])
        nc.scalar.mul(R[0:3, :], R[0:3, :], 2.0)
        nc.sync.dma_start(R[3:4, :], consts_d[0:1, 0:Ng])
        Cg = Ng // 128
        yb = tmpp.tile([128, 3 * Cg], FP32, tag="yb")
        # [p, k, c] = y-coord k of point p*Cg+c  (p-major point blocks)
        nc.sync.dma_start(yb[:], gt_d.rearrange("k (p c) -> p k c", p=128))
        nc.scalar.activation(yb[:], yb[:], mybir.ActivationFunctionType.Square)
        ysum = tmpp.tile([128, Cg], FP32, tag="ysum")
        ybv = yb.rearrange("p (k c) -> p k c", k=3)
        nc.vector.tensor_tensor(ysum[:], ybv[:, 0, :], ybv[:, 1, :], op=OP.add)
        nc.vector.tensor_tensor(ysum[:], ysum[:], ybv[:, 2, :], op=OP.add)
        nc.sync.dma_start(R[4:5, :], ysum[:])
        for q in (32, 64, 96):
            nc.sync.dma_start(R[q : q + 5, :], R[0:5, :])

        # ---- build L = [2x0;2x1;2x2; -|x|^2; -1] for fine and coarse ----
        def build_L(src_d, n):
            # src_d is [3, n] coordinate-major (host pre-transposed)
            Lt = lr.tile([128, n], FP32, tag=f"L{n}")
            nc.sync.dma_start(Lt[0:3, :], src_d[:, :])
            nc.sync.dma_start(Lt[4:5, :], consts_d[1:2, 0:n])
            C = n // 128
            xb = tmpp.tile([128, 3 * C], FP32, tag="yb")
            # [p, k, c] = coord k of point p*C+c  (p-major point blocks)
            nc.sync.dma_start(xb[:], src_d.rearrange("k (p c) -> p k c", p=128))
            nc.scalar.activation(xb[:], xb[:], mybir.ActivationFunctionType.Square)
            xsum = tmpp.tile([128, C], FP32, tag="ysum")
            xbv = xb.rearrange("p (k c) -> p k c", k=3)
            nc.vector.tensor_tensor(xsum[:], xbv[:, 0, :], xbv[:, 1, :], op=OP.add)
            nc.vector.tensor_tensor(xsum[:], xsum[:], xbv[:, 2, :], op=OP.add)
            nc.scalar.mul(xsum[:], xsum[:], -1.0)
            nc.sync.dma_start(Lt[3:4, :], xsum[:])
            for q in (32, 64, 96):
                nc.sync.dma_start(Lt[q : q + 5, :], Lt[0:5, :])
            return Lt

        Lf = build_L(fine_d, Nf)
        Lc = build_L(coarse_d, Ncs)

        # ---- accumulators ----
        colacc_f = accp.tile([128, Ng], FP16)
        colacc_c = accp.tile([128, Ng], FP16)
        rmins_f = accp.tile([128, Nf // 128], FP32)
        rmins_c = accp.tile([128, Ncs // 128], FP32)

        # ---- main pairwise pass ----
        with tc.tile_pool(name="psum", bufs=2, space="PSUM") as psum_pool:

            def cloud_pass(Lt, n_xt, colacc, rmins):
                for mi in range(n_xt):
                    scr_x = scrp.tile([128, Ng], FP16, tag="scrx")
                    for g in range(n_groups):
                        ps = psum_pool.tile([128, group], FP32, tag="ps")
                        for j in range(group // mm_n):
                            ny0 = g * group + j * mm_n
                            q = 32 * j
                            nc.tensor.matmul(
                                ps[:, j * mm_n : (j + 1) * mm_n],
                                Lt[q : q + 5, mi * 128 : (mi + 1) * 128],
                                R[q : q + 5, ny0 : ny0 + mm_n],
                                start=True,
                                stop=True,
                                tile_position=(q, 0),
                            )
                        # ScalarE drains PSUM to fp16 SBUF; DVE runs in fast
                        # 16-bit SBUF modes only.
                        gs = slice(g * group, (g + 1) * group)
                        nc.scalar.copy(scr_x[:, gs], ps[:])
                        if g % 2 == 1 or g == n_groups - 1:
                            # update colacc in up-to-4096-wide ops (fewer DVE
                            # ops, still overlaps the next group's ACT copy)
                            lo = (g - 1 if g % 2 == 1 else g) * group
                            g2 = slice(lo, (g + 1) * group)
                            if mi == 0:
                                nc.vector.tensor_copy(colacc[:, g2], scr_x[:, g2])
                            else:
                                nc.vector.tensor_tensor(
                                    colacc[:, g2],
                                    colacc[:, g2],
                                    scr_x[:, g2],
                                    op=OP.max,
                                )
                    # row direction: in-place halving fold tree (2x fp16 TT),
                    # then one 1x reduce of the last 512 columns.
                    w = Ng
                    while w > 512:
                        w //= 2
                        nc.vector.tensor_tensor(
                            scr_x[:, 0:w],
                            scr_x[:, 0:w],
                            scr_x[:, w : 2 * w],
                            op=OP.max,
                        )
                    nc.vector.reduce_max(
                        rmins[:, mi : mi + 1], scr_x[:, 0:w], axis=AX.X
                    )

            cloud_pass(Lf, Nf // 128, colacc_f, rmins_f)
            cloud_pass(Lc, Ncs // 128, colacc_c, rmins_c)

        # ---- column direction: partition-reduce colacc via PE transposes ----
        cmaxs_f = accp.tile([128, Ng // 128], FP32)
        cmaxs_c = accp.tile([128, Ng // 128], FP32)
        with tc.tile_pool(name="tpsum", bufs=4, space="PSUM") as tpsum:
            for colacc, cmaxs in ((colacc_f, cmaxs_f), (colacc_c, cmaxs_c)):
                for k in range(Ng // 128):
                    tp = tpsum.tile([128, 128], FP16, tag="tp")
                    nc.tensor.transpose(
                        tp[:], colacc[:, k * 128 : (k + 1) * 128], ident[:]
                    )
                    nc.vector.reduce_max(cmaxs[:, k : k + 1], tp[:], axis=AX.X)

        # ---- total sums -> 4 scalars ----
        # free-axis sums into a [128,4] matrix, then one K=128 matmul with a
        # ones vector does all four partition-axis sums at once.
        T4 = smallp.tile([128, 4], FP32)
        for idx, mat in enumerate((rmins_f, cmaxs_f, rmins_c, cmaxs_c)):
            nc.vector.reduce_sum(T4[:, idx : idx + 1], mat[:], axis=AX.X)
        ones_col = const.tile([128, 1], FP32)
        nc.vector.memset(ones_col[:], 1.0)
        with tc.tile_pool(name="fpsum", bufs=1, space="PSUM") as fpsum:
            outp = fpsum.tile([1, 4], FP32)
            nc.tensor.matmul(outp[:], ones_col[:], T4[:], start=True, stop=True)
            nc.vector.tensor_copy(out_sb[:], outp[:])

        nc.sync.dma_start(out_d[:, :], out_sb[:])

    return nc


def _register_ntff_hook():
    """The agent image's antenv lacks axon_hooks; synthesize the module and
    register the ctypes NTFF hook from trn_agent_boot so trace=True works."""
    import types

    try:
        from antenv import axon_hooks  # noqa: F401

        return True
    except ImportError:
        pass
    try:
        import antenv

        sys.path.insert(0, "/root/.axon_site")
        from trn_agent_boot.trn_boot import _ntff_profile_via_ctypes

        hook = _ntff_profile_via_ctypes("/opt/axon/libaxon_pjrt.so")
        mod = types.ModuleType("antenv.axon_hooks")
        _state = {"hook": hook}
        mod.set_axon_ntff_profile_hook = lambda h: _state.__setitem__("hook", h)
        mod.get_axon_ntff_profile_hook = lambda: _state["hook"]
        sys.modules["antenv.axon_hooks"] = mod
        antenv.axon_hooks = mod
        return hook is not None
    except Exception as e:  # profiling is best-effort
        print(f"ntff hook registration failed: {e}", file=sys.stderr)
        return False


_COMPILED = None


def _get_program():
    global _COMPILED
    if _COMPILED is None:
        nc = bass.Bass()
        emit_chamfer(nc, NF, NG, NC)
        cap_sync_waits(nc)
        _COMPILED = nc
    return _COMPILED


def kernel(coarse, fine, gt, alpha):
    global LAST_EXEC_NS
    from concourse.bass_utils import run_bass_kernel_spmd

    coarse = np.asarray(coarse, dtype=np.float32)
    fine = np.asarray(fine, dtype=np.float32)
    gt = np.asarray(gt, dtype=np.float32)
    alpha = np.asarray(alpha, dtype=np.float32)

    ident = np.eye(128, dtype=np.float16)
    consts = np.empty((2, max(NF, NG)), dtype=np.float32)
    consts[0] = 1.0
    consts[1] = -1.0
    in_maps = [
        {
            "fineT": np.ascontiguousarray(fine[b].T),
            "gt": np.ascontiguousarray(gt[b]),
            "coarseT": np.ascontiguousarray(coarse[b].T),
            "ident": ident,
            "consts": consts,
        }
        for b in range(B)
    ]

    nc = _get_program()
    trace = bool(int(os.environ.get("CHAMFER_TRACE", "0")))
    if trace:
        trace = _register_ntff_hook()
    res = run_bass_kernel_spmd(nc, in_maps, list(range(B)), trace=trace)
    if trace:
        LAST_EXEC_NS = res.exec_time_ns

    loss_fine_b = np.empty(B, dtype=np.float64)
    loss_coarse_b = np.empty(B, dtype=np.float64)
    for b in range(B):
        s = res.results[b]["out"].astype(np.float64).ravel()
        # s = [sum rowmax(-d) fine, sum colmax(-d) fine,
        #      sum rowmax(-d) coarse, sum colmax(-d) coarse]
        loss_fine_b[b] = -(s[0] / NF + s[1] / NG)
        loss_coarse_b[b] = -(s[2] / NC + s[3] / NG)

    loss_fine = loss_fine_b.mean()
    loss_coarse = loss_coarse_b.mean()
    loss = loss_coarse + float(alpha[0]) * loss_fine
    return (
        np.float32(loss),
        np.float32(loss_coarse),
        np.float32(loss_fine),
    )
